# revision 1
# baseline (speedup 1.0000x reference)
"""CBOW forward (embedding lookup -> linear -> log_softmax) on 8 TRN2 NeuronCores.

Problem: nn_CBOW_49701361549346
  input_vec_list [2N=8, B=256, V=50000] f32 one-hot context vectors
  w1 [64, 50000], b1 [64], w2 [50000, 64], b2 [50000]
  out = log_softmax((mean_i x_i) @ w1.T + b1) @ w2.T + b2, axis=-1) -> [256, 50000] f32

Strategy (data-parallel over batch, 32 rows/core):
  - Host: collapse the one-hot vectors to (index, value) pairs -- they carry
    2048 ints of information; reading 410 MB of zeros on-device would dominate.
    Pre-transpose w1 -> [V, 64] so the device-side embedding gather is
    contiguous rows; pack w2.T with b2 appended as a 65th contraction row in
    fp8e4 (logits are ~1e-2 scale on an 10.8-magnitude output -- fp8 error is
    ~1e-3 absolute, far inside tolerance), columns permuted to the device
    (group, quarter) tiling.
  - Device (identical program on all 8 cores, per-core inputs):
      1. indirect-DMA gather of the 8*32 = 256 context embedding rows
      2. h^T = G^T @ SEL (SEL folds the 1/8 mean, per-row one-hot value, and
         batch regroup), + b1 -> fp8 [65, 32] with a ones row (bias trick)
      3. logits: 4-bank PSUM supertiles, 16 matmuls each -- 4 vocab quarters
         stacked across partitions x 4 bank-aligned column groups; the four
         col_grp matmuls stream concurrently through separate XBUSes
      4. ScalarE exp + accumulated row-sums per supertile; VectorE casts
         bf16 logits into a resident SBUF store
      5. cross-quarter sum via a small selection matmul, ln -> -logZ
      6. pass 2: out = logits - logZ (ScalarE bias-add) -> one 1.28 MB DMA
         per output chunk (3D access pattern covering all 4 quarters)
"""

import numpy as np
import ml_dtypes

import concourse.bass as bass
import concourse.bacc as bacc
import concourse.mybir as mybir
import concourse.tile as tile
from concourse.bass_utils import run_bass_kernel_spmd

# Problem constants (hardcoded per contract)
NCTX = 8          # 2N context positions
B = 256           # batch
V = 50000         # vocab
D = 64            # embed dim
NCORES = 8
BS = B // NCORES  # 32 batch rows per core

VQ = V // 4       # 12500, vocab quarter held per partition-group
GW = 500          # columns per quarter per group (one psum bank, 512-aligned)
NG = VQ // GW     # 25 groups; each group covers 4*GW = 2000 vocab columns
SGG = 4           # groups per psum supertile (4 banks = [128, 2048])
OW = 1250         # pass-2 output chunk width (per quarter)
NO = VQ // OW     # 10 output chunks (alternating ScalarE/VectorE)

F32 = mybir.dt.float32
BF16 = mybir.dt.bfloat16
F16 = mybir.dt.float16
FP8 = mybir.dt.float8e4
I32 = mybir.dt.int32
FP8_NP = ml_dtypes.float8_e4m3

_CACHE = {}


def _build_bass():
    """Build the single-core Bass program (same NEFF runs SPMD on all cores)."""
    nc = bacc.Bacc("TRN2", target_bir_lowering=False, debug=False, num_devices=NCORES)

    # NOTE: indirect-DMA offset tiles must be partition-0 based (a
    # partition-offset offset tile broke at runtime on HW), and gathers
    # serialize on the gpsimd queue -- two 128-row gathers is the sweet spot.
    idx_d = nc.dram_tensor("idx", [128, 2], I32, kind="ExternalInput")
    # sel[p, t*32 + m] = val(p, t)/8 if p % 32 == m else 0: folds the context
    # mean, the per-row one-hot value, and the batch regroup into layer 1.
    sel_d = nc.dram_tensor("sel", [128, 2 * BS], F32, kind="ExternalInput")
    b1_d = nc.dram_tensor("b1", [D], F32, kind="ExternalInput")
    w1t_d = nc.dram_tensor("w1t", [V, D], F32, kind="ExternalInput")
    w2te_d = nc.dram_tensor("w2te", [D + 1, V], FP8, kind="ExternalInput")
    # output in the device's native (quarter, batch)-partition layout:
    # out[q*32+b, c] = logits[b, q*12500 + c]; the host unpermutes during
    # unsharding. Keeps every output DMA a clean 2D partition-major transfer.
    out_d = nc.dram_tensor("out", [128, VQ], F16, kind="ExternalOutput")

    # QSEL[k, p] = 1 if k % 32 == p % 32 : sums the 4 vocab quarters per batch
    # row and broadcasts the result to all 128 partitions in one matmul.
    qsel_np = (np.arange(128)[:, None] % BS == np.arange(128)[None, :] % BS)
    qsel_d = nc.inline_tensor(qsel_np.astype(np.float32), name="qsel_const")

    # supergroup schedule: (start_group, n_groups) per psum supertile
    sgs = []
    g0 = 0
    while g0 < NG:
        sgs.append((g0, min(SGG, NG - g0)))
        g0 += SGG

    with tile.TileContext(nc) as tc:
        with (
            tc.tile_pool(name="consts", bufs=1) as consts,
            tc.tile_pool(name="gather", bufs=4) as gather,
            tc.tile_pool(name="wpool", bufs=3) as wpool,
            tc.tile_pool(name="logits", bufs=1) as logits,
            tc.tile_pool(name="scratch", bufs=2) as scratch,
            tc.tile_pool(name="stats", bufs=1) as stats,
            tc.tile_pool(name="opool", bufs=3) as opool,
            tc.tile_pool(name="psum", bufs=2, space="PSUM") as psum,
        ):
            # setup loads on the scalar (ACT) HWDGE ring -- keeps the sync
            # ring free for the big streaming transfers, and gets idx (which
            # gates the gather -> h -> everything) out first.
            idx_sb = consts.tile([128, 2], I32)
            nc.scalar.dma_start(out=idx_sb[:], in_=idx_d[:])
            sel_sb = consts.tile([128, 2 * BS], F32)
            nc.scalar.dma_start(out=sel_sb[:], in_=sel_d[:])
            b1_sb = consts.tile([D, 1], F32)
            nc.scalar.dma_start(out=b1_sb[:], in_=b1_d[:, None])
            qsel_sb = consts.tile([128, 128], F32)
            nc.scalar.dma_start(out=qsel_sb[:], in_=qsel_d[:])

            # ---- layer 1: gather context embeddings, reduce to h^T [64, 32]
            hT_ps = psum.tile([128, SGG * 512], F32, tag="ps")
            for t in range(2):
                g = gather.tile([128, D], F32)
                nc.gpsimd.indirect_dma_start(
                    out=g[:],
                    out_offset=None,
                    in_=w1t_d[:],
                    in_offset=bass.IndirectOffsetOnAxis(
                        ap=idx_sb[:, t : t + 1], axis=0
                    ),
                )
                nc.tensor.matmul(
                    hT_ps[:D, :BS],
                    lhsT=g[:],
                    rhs=sel_sb[:, t * BS : (t + 1) * BS],
                    start=(t == 0),
                    stop=(t == 1),
                )

            # hT_ext [65, 32] fp8: rows 0..63 = h^T + b1, row 64 = 1.0 (b2 row)
            hT = consts.tile([D + 1, BS], FP8)
            nc.vector.memset(hT[D : D + 1, :], 1.0)
            nc.scalar.activation(
                hT[0:D, :], hT_ps[:D, :BS], mybir.ActivationFunctionType.Identity,
                bias=b1_sb[:, 0:1], scale=1.0,
            )

            # ---- layer 2 phase 1: logits supertiles, exp row-sums, bf16 store
            L = logits.tile([128, VQ], F16)           # logits store, 25 KB/partition
            s_part = stats.tile([128, len(sgs)], F32)  # per-supertile exp sums
            for si, (gs, ng) in enumerate(sgs):
                wt = wpool.tile([D + 1, SGG * 4 * GW], FP8, tag="wt")
                nc.sync.dma_start(
                    out=wt[:, : ng * 4 * GW],
                    in_=w2te_d[:, gs * 4 * GW : (gs + ng) * 4 * GW],
                )
                pg = psum.tile([128, SGG * 512], F32, tag="ps")
                for j in range(ng):
                    for q in range(4):
                        nc.tensor.matmul(
                            pg[q * BS : (q + 1) * BS, j * 512 : j * 512 + GW],
                            lhsT=hT[:],
                            rhs=wt[:, (j * 4 + q) * GW : (j * 4 + q + 1) * GW],
                            start=True,
                            stop=True,
                            tile_position=(0, q * BS),
                        )
                # strided view excluding the 12-col bank padding
                pg_v = pg[:, : ng * 512].rearrange("p (g x) -> p g x", x=512)[:, :, :GW]
                # VectorE is the only psum consumer (frees the bank fast);
                # ScalarE exps from the bf16 store (error ~logit rounding, tiny)
                lsl = L[:, gs * GW : (gs + ng) * GW]
                nc.vector.tensor_copy(
                    lsl.rearrange("p (g x) -> p g x", x=GW), pg_v
                )
                e = scratch.tile([128, SGG * GW], F32, tag="e")
                nc.scalar.activation(
                    e[:, : ng * GW], lsl, mybir.ActivationFunctionType.Exp,
                    accum_out=s_part[:, si : si + 1],
                )

            # ---- logZ per batch row, broadcast to all 128 partitions
            s1 = stats.tile([128, 1], F32)
            nc.vector.reduce_sum(s1[:], s_part[:], axis=mybir.AxisListType.X)
            z_ps = psum.tile([128, SGG * 512], F32, tag="ps")
            nc.tensor.matmul(
                z_ps[:, 0:1], lhsT=qsel_sb[:], rhs=s1[:], start=True, stop=True
            )
            negc = stats.tile([128, 1], F32)
            nc.scalar.activation(negc[:], z_ps[:, 0:1], mybir.ActivationFunctionType.Ln)
            nc.vector.tensor_scalar_mul(negc[:], negc[:], -1.0)

            # ---- pass 2: out = logits - logZ, partition-major DMAs.
            # chunks alternate ScalarE / VectorE so the subtract feeds the
            # output stream at 2x one engine's rate
            for oi in range(NO):
                o = opool.tile([128, OW], F16)
                lsl = L[:, oi * OW : (oi + 1) * OW]
                if oi % 2 == 0:
                    nc.scalar.activation(
                        o[:], lsl, mybir.ActivationFunctionType.Identity,
                        bias=negc[:, 0:1], scale=1.0,
                    )
                else:
                    nc.vector.tensor_scalar_add(o[:], lsl, negc[:, 0:1])
                nc.sync.dma_start(
                    out=out_d[:, oi * OW : (oi + 1) * OW], in_=o[:]
                )

    nc.finalize()
    return nc


def _prep_shared(w1, b1, w2, b2):
    w1t = np.ascontiguousarray(w1.T).astype(np.float32, copy=False)   # [V, 64]
    w2te = np.concatenate(
        [w2.T.astype(np.float32, copy=False), b2[None, :].astype(np.float32, copy=False)],
        axis=0,
    )  # [65, V]
    # permute columns: v = q*VQ + g*GW + j  ->  c = g*4*GW + q*GW + j
    w2te = np.ascontiguousarray(
        w2te.reshape(D + 1, 4, NG, GW).transpose(0, 2, 1, 3).reshape(D + 1, V)
    ).astype(FP8_NP)
    return w1t, w2te, np.ascontiguousarray(b1).astype(np.float32, copy=False)


def _make_in_maps(input_vec_list, w1, b1, w2, b2):
    x = np.asarray(input_vec_list)
    assert x.shape == (NCTX, B, V), x.shape

    # Collapse one-hot context vectors to (index, value) pairs on the host.
    ids = np.argmax(x, axis=-1).astype(np.int32)          # [8, 256]
    vals = np.max(x, axis=-1).astype(np.float32)          # [8, 256] (0 for all-zero rows)

    w1t, w2te, b1c = _prep_shared(
        np.asarray(w1), np.asarray(b1), np.asarray(w2), np.asarray(b2)
    )

    # per-core layout: tile t row p <-> (i = 4t + p//32, b = c*32 + p%32)
    i_of_p = np.arange(128) // BS
    b_of_p = np.arange(128) % BS
    in_maps = []
    for c in range(NCORES):
        idx_core = np.zeros((128, 2), dtype=np.int32)
        sel_core = np.zeros((128, 2 * BS), dtype=np.float32)
        for t in range(2):
            idx_core[:, t] = ids[4 * t + i_of_p, c * BS + b_of_p]
            sel_core[np.arange(128), t * BS + b_of_p] = (
                vals[4 * t + i_of_p, c * BS + b_of_p] / NCTX
            )
        in_maps.append(
            {"idx": idx_core, "sel": sel_core, "b1": b1c, "w1t": w1t, "w2te": w2te}
        )
    return in_maps


def _get_nc():
    if "nc" not in _CACHE:
        _CACHE["nc"] = _build_bass()
    return _CACHE["nc"]


def _unpermute(res_core):
    """[128, VQ] (q,b)-partition layout (f16) -> [32, V] f32 batch rows."""
    return np.ascontiguousarray(
        res_core.astype(np.float32).reshape(4, BS, VQ).transpose(1, 0, 2).reshape(BS, V)
    )


def kernel(input_vec_list, w1, b1, w2, b2):
    in_maps = _make_in_maps(input_vec_list, w1, b1, w2, b2)
    res = run_bass_kernel_spmd(_get_nc(), in_maps, list(range(NCORES)))
    out = np.concatenate(
        [_unpermute(res.results[c]["out"]) for c in range(NCORES)], axis=0
    )
    return out.astype(np.float32, copy=False)

